# revision 20
# baseline (speedup 1.0000x reference)
"""NeuroPredessor GNN message-passing kernel for 8 Trainium2 NeuronCores.

Sharding: core c owns vars [512c, 512c+512) and nodes [1024c, 1024c+1024).
The sparse unpack matrix is densified host-side to bf16 (integer counts are
exact in bf16) into two SBUF-resident shards per core:
  - cs: unpack[:, node_shard]            [4096, 1024]  (rhs of c2p SpMM)
  - rs: unpack[var_shard, :].T           [8192,  512]  (rhs of p2c SpMM)
Activations are feature-major [128(dim), tokens]; msg-MLP layer 3 emits
token-major tiles that are AllGathered (bf16) and feed the SpMM as lhsT.
The msg l3 bias is folded into the SpMM as one extra rank: b3 (x) deg.
"""

import sys

for _p in ("/opt/trn_rl_repo", "/root/.axon_site/_ro/trn_rl_repo"):
    if _p not in sys.path:
        sys.path.append(_p)

import numpy as np
import ml_dtypes

import concourse.bass as bass
import concourse.mybir as mybir
from concourse import tile, bacc
from concourse.bass_utils import run_bass_kernel_spmd

BF16 = mybir.dt.bfloat16
F32 = mybir.dt.float32
AF = mybir.ActivationFunctionType

N_VAR = 4096
N_NODE = 8192
DIM = 128
N_ROUNDS = 16
N_CORES = 8
VS = N_VAR // N_CORES  # 512 vars per core
NS = N_NODE // N_CORES  # 1024 nodes per core
KV = N_VAR // 128  # 32 var k-tiles
KN = N_NODE // 128  # 64 node k-tiles
P = 128

_nbf = ml_dtypes.bfloat16


def _build_nc(n_rounds=N_ROUNDS):
    nc = bacc.Bacc(None)

    # ---------------- external inputs ----------------
    cs_d = nc.dram_tensor("cs", [KV + 1, P, NS], BF16, kind="ExternalInput")
    rs_d = nc.dram_tensor("rs", [KN + 1, P, VS], BF16, kind="ExternalInput")
    wmv_d = nc.dram_tensor("wmv", [P, 3, P], BF16, kind="ExternalInput")
    wmn_d = nc.dram_tensor("wmn", [P, 3, P], BF16, kind="ExternalInput")
    wnih_d = nc.dram_tensor("wnih", [P, 4 * P], BF16, kind="ExternalInput")
    wnhh_d = nc.dram_tensor("wnhh", [P, 4 * P], BF16, kind="ExternalInput")
    wvih_d = nc.dram_tensor("wvih", [P, 4 * P], BF16, kind="ExternalInput")
    wvhh_d = nc.dram_tensor("wvhh", [P, 4 * P], BF16, kind="ExternalInput")
    wvo_d = nc.dram_tensor("wvo", [P, 3, P], BF16, kind="ExternalInput")
    b3v_d = nc.dram_tensor("b3v", [P, P], BF16, kind="ExternalInput")
    b3n_d = nc.dram_tensor("b3n", [P, P], BF16, kind="ExternalInput")
    bias_d = nc.dram_tensor("bias", [P, 16], F32, kind="ExternalInput")
    # bias cols: 0..1 var_msg b1,b2 | 2..3 node_msg b1,b2 | 4..7 node lstm i,f,g,o
    #            8..11 var lstm i,f,g,o | 12..13 vote b1,b2 | 14 vote b3 (row0)
    vh0_d = nc.dram_tensor("vh0", [P, VS], BF16, kind="ExternalInput")
    nh0_d = nc.dram_tensor("nh0", [P, NS], BF16, kind="ExternalInput")

    y_d = nc.dram_tensor("y", [1, VS], F32, kind="ExternalOutput")

    # ---------------- collective bounce buffers ----------------
    # var AG is split into two 512KB chunks (token halves) so nrt picks the
    # low-latency Mesh algorithm (<1MB) and chunk A's SpMM overlaps chunk B.
    vp_in = [
        [nc.dram_tensor(f"vp_in_{r}_{a}", [VS // 2, P], BF16) for a in range(2)]
        for r in range(n_rounds)
    ]
    vp_out = [
        [
            nc.dram_tensor(f"vp_out_{r}_{a}", [N_VAR // 2, P], BF16, addr_space="Shared")
            for a in range(2)
        ]
        for r in range(n_rounds)
    ]
    # node AG likewise split into four 512KB Mesh chunks (256-token slices).
    np_in = [
        [nc.dram_tensor(f"np_in_{r}_{a}", [NS // 4, P], BF16) for a in range(4)]
        for r in range(n_rounds)
    ]
    np_out = [
        [
            nc.dram_tensor(f"np_out_{r}_{a}", [N_NODE // 4, P], BF16, addr_space="Shared")
            for a in range(4)
        ]
        for r in range(n_rounds)
    ]

    RG = [list(range(N_CORES))]

    with tile.TileContext(nc) as tc:
        with (
            tc.tile_pool(name="const", bufs=1) as cpool,
            tc.tile_pool(name="state", bufs=1) as spool,
            tc.tile_pool(name="vp", bufs=16) as vppool,
            tc.tile_pool(name="npp", bufs=10) as nppool,
            tc.tile_pool(name="work", bufs=3) as wpool,
            tc.tile_pool(name="lstm", bufs=7) as lpool,
            tc.tile_pool(name="stage", bufs=6) as stpool,
            tc.tile_pool(name="psA", bufs=6, space="PSUM") as psA,
            tc.tile_pool(name="psB", bufs=2, space="PSUM") as psB,
        ):
            # ---------- resident loads ----------
            # Small weight/state loads go first (round 0 needs them); the big
            # unpack shards stream behind them, spread over two DMA queues.
            def cload(shape, dt, dram, nm):
                t = cpool.tile(shape, dt, name=nm)
                nc.sync.dma_start(t[:], dram[:])
                return t

            wmv = cload([P, 3, P], BF16, wmv_d, "wmv_sb")
            wmn = cload([P, 3, P], BF16, wmn_d, "wmn_sb")
            wnih = cload([P, 4 * P], BF16, wnih_d, "wnih_sb")
            wnhh = cload([P, 4 * P], BF16, wnhh_d, "wnhh_sb")
            wvih = cload([P, 4 * P], BF16, wvih_d, "wvih_sb")
            wvhh = cload([P, 4 * P], BF16, wvhh_d, "wvhh_sb")
            wvo = cload([P, 3, P], BF16, wvo_d, "wvo_sb")
            b3v = cload([P, P], BF16, b3v_d, "b3v_sb")
            b3n = cload([P, P], BF16, b3n_d, "b3n_sb")
            bias = cload([P, 16], F32, bias_d, "bias_sb")

            # persistent states (in-place updated)
            var_h = spool.tile([P, VS], BF16, name="var_h")
            nc.sync.dma_start(var_h[:], vh0_d[:])
            node_h = spool.tile([P, NS], BF16, name="node_h")
            nc.sync.dma_start(node_h[:], nh0_d[:])
            var_c = spool.tile([P, VS], F32, name="var_c")
            nc.vector.memset(var_c[:], 0.0)
            node_c = spool.tile([P, NS], F32, name="node_c")
            nc.vector.memset(node_c[:], 0.0)

            cs = cpool.tile([P, KV + 1, NS], BF16, name="cs_sb")
            for k in range(KV + 1):
                (nc.sync if k % 2 == 0 else nc.scalar).dma_start(cs[:, k, :], cs_d[k])
            rs = cpool.tile([P, KN + 1, VS], BF16, name="rs_sb")
            for k in range(KN + 1):
                (nc.sync if k % 2 == 0 else nc.scalar).dma_start(rs[:, k, :], rs_d[k])

            def mlp2(w, x, b0, ntok, nm):
                """Two relu layers, feature-major bf16 in/out."""
                h = x
                for li in range(2):
                    hn = wpool.tile([P, ntok], BF16, tag=f"w{ntok}", name=f"{nm}_h{li}")
                    for t0 in range(0, ntok, 512):
                        ps = psA.tile([P, 512], F32, tag="mm", name=f"{nm}_l{li}_{t0}")
                        nc.tensor.matmul(
                            ps[:], w[:, li, :], h[:, t0 : t0 + 512],
                            start=True, stop=True,
                        )
                        nc.scalar.activation(
                            hn[:, t0 : t0 + 512], ps[:], AF.Relu,
                            bias=bias[:, b0 + li : b0 + li + 1],
                        )
                    h = hn
                return h

            dma_engs = [nc.sync, nc.scalar]

            def msg_l3_to_bounce(w, h2, ntok, dram, nm):
                """l3 (no bias), token-major out, DMA into collective bounce."""
                for t in range(ntok // P):
                    ps = psB.tile([P, P], F32, tag="tok", name=f"{nm}_t{t}")
                    nc.tensor.matmul(
                        ps[:], h2[:, t * P : (t + 1) * P], w[:, 2, :],
                        start=True, stop=True,
                    )
                    st = stpool.tile([P, P], BF16, tag="st", name=f"{nm}_s{t}")
                    nc.scalar.activation(st[:], ps[:], AF.Copy)
                    dma_engs[t % 2].dma_start(dram[t * P : (t + 1) * P, :], st[:])

            def lstm_half(gps, c_st, h_st, hsl, b0, nm):
                """Gate psums [i,f,g,o] -> in-place update c_st/h_st slices."""
                i_s = lpool.tile([P, 512], F32, tag="ls", name=f"{nm}_i")
                f_s = lpool.tile([P, 512], F32, tag="ls", name=f"{nm}_f")
                g_t = lpool.tile([P, 512], F32, tag="ls", name=f"{nm}_g")
                o_s = lpool.tile([P, 512], F32, tag="ls", name=f"{nm}_o")
                nc.scalar.activation(i_s[:], gps[0][:], AF.Sigmoid, bias=bias[:, b0 : b0 + 1])
                nc.scalar.activation(f_s[:], gps[1][:], AF.Sigmoid, bias=bias[:, b0 + 1 : b0 + 2])
                nc.scalar.activation(g_t[:], gps[2][:], AF.Tanh, bias=bias[:, b0 + 2 : b0 + 3])
                nc.scalar.activation(o_s[:], gps[3][:], AF.Sigmoid, bias=bias[:, b0 + 3 : b0 + 4])
                t1 = lpool.tile([P, 512], F32, tag="ls", name=f"{nm}_t1")
                nc.vector.tensor_mul(t1[:], i_s[:], g_t[:])
                t2 = lpool.tile([P, 512], F32, tag="ls", name=f"{nm}_t2")
                nc.vector.tensor_mul(t2[:], f_s[:], c_st[:, hsl])
                nc.vector.tensor_add(c_st[:, hsl], t1[:], t2[:])
                tc2 = lpool.tile([P, 512], F32, tag="ls", name=f"{nm}_tc")
                nc.scalar.activation(tc2[:], c_st[:, hsl], AF.Tanh)
                nc.vector.tensor_mul(h_st[:, hsl], o_s[:], tc2[:])

            for r in range(n_rounds):
                # ===== var msg MLP + l3 -> bounce =====
                h2 = mlp2(wmv, var_h, 0, VS, f"r{r}_vm")
                for a in range(2):
                    for t in range(2):
                        ps = psB.tile([P, P], F32, tag="tok", name=f"r{r}_vm3_{a}{t}")
                        tt = 2 * a + t
                        nc.tensor.matmul(
                            ps[:], h2[:, tt * P : (tt + 1) * P], wmv[:, 2, :],
                            start=True, stop=True,
                        )
                        st = stpool.tile([P, P], BF16, tag="st", name=f"r{r}_vm3s_{a}{t}")
                        nc.scalar.activation(st[:], ps[:], AF.Copy)
                        dma_engs[t % 2].dma_start(vp_in[r][a][t * P : (t + 1) * P, :], st[:])
                    nc.gpsimd.collective_compute(
                        "AllGather", mybir.AluOpType.bypass, replica_groups=RG,
                        ins=[vp_in[r][a][:]], outs=[vp_out[r][a][:]],
                    )
                # ===== SpMM1 + node LSTM, sequential halves =====
                # Per half: bias-rank and Whh-part matmuls are emitted BEFORE
                # the k-loop so they sit ahead in the PE queue and execute
                # while the AllGather is still in flight. The k order consumes
                # AG chunk 0's tiles first so they overlap chunk 1's flight.
                vp_views = [
                    vp_out[r][a][:].rearrange("(g j p) d -> g p j d", j=2, p=P)
                    for a in range(2)
                ]
                k_order = [4 * g + 2 * a + j for a in range(2) for j in range(2)
                           for g in range(KV // 4)]
                vpt = None
                for h in range(2):
                    hsl = slice(h * 512, h * 512 + 512)
                    c2p = psA.tile([P, 512], F32, tag="mm", name=f"r{r}_c2p{h}")
                    nc.tensor.matmul(c2p[:], b3v[:], cs[:, KV, hsl],
                                     start=True, stop=False)
                    gps = []
                    for g in range(4):
                        ps = psA.tile([P, 512], F32, tag="mm", name=f"r{r}_ng{h}{g}")
                        nc.tensor.matmul(ps[:], wnhh[:, g * P : (g + 1) * P],
                                         node_h[:, hsl], start=True, stop=False)
                        gps.append(ps)
                    if vpt is None:
                        vpt = {}
                        for a in range(2):
                            for g in range(KV // 4):
                                t = vppool.tile([P, 2, P], BF16, tag="vp",
                                                name=f"r{r}_vp{a}_{g}")
                                dma_engs[g % 2].dma_start(t[:], vp_views[a][g])
                                vpt[(a, g)] = t
                    for i, k in enumerate(k_order):
                        a, j, g = (k % 4) // 2, (k % 4) % 2, k // 4
                        nc.tensor.matmul(c2p[:], vpt[(a, g)][:, j, :],
                                         cs[:, k, hsl],
                                         start=False, stop=(i == KV - 1))
                    x_sb = wpool.tile([P, 512], BF16, tag="w512", name=f"r{r}_c2ps{h}")
                    nc.vector.tensor_copy(x_sb[:], c2p[:])
                    for g in range(4):
                        nc.tensor.matmul(gps[g][:], wnih[:, g * P : (g + 1) * P],
                                         x_sb[:], start=False, stop=True)
                    lstm_half(gps, node_c, node_h, hsl, 4, f"r{r}_nl{h}")

                # ===== node msg MLP + l3 -> chunked bounce/AG =====
                h2n = mlp2(wmn, node_h, 2, NS, f"r{r}_nm")
                for a in range(4):
                    for t in range(2):
                        ps = psB.tile([P, P], F32, tag="tok", name=f"r{r}_nm3_{a}{t}")
                        tt = 2 * a + t
                        nc.tensor.matmul(
                            ps[:], h2n[:, tt * P : (tt + 1) * P], wmn[:, 2, :],
                            start=True, stop=True,
                        )
                        st = stpool.tile([P, P], BF16, tag="st", name=f"r{r}_nm3s_{a}{t}")
                        nc.scalar.activation(st[:], ps[:], AF.Copy)
                        dma_engs[t % 2].dma_start(np_in[r][a][t * P : (t + 1) * P, :], st[:])
                    nc.gpsimd.collective_compute(
                        "AllGather", mybir.AluOpType.bypass, replica_groups=RG,
                        ins=[np_in[r][a][:]], outs=[np_out[r][a][:]],
                    )
                # ===== SpMM2 + var LSTM (bias/Whh emitted first for AG overlap) =====
                p2c_ps = psA.tile([P, 512], F32, tag="mm", name=f"r{r}_p2c")
                nc.tensor.matmul(p2c_ps[:], b3n[:], rs[:, KN, :],
                                 start=True, stop=False)
                gps = []
                for g in range(4):
                    ps = psA.tile([P, 512], F32, tag="mm", name=f"r{r}_vg{g}")
                    nc.tensor.matmul(ps[:], wvhh[:, g * P : (g + 1) * P], var_h[:],
                                     start=True, stop=False)
                    gps.append(ps)
                np_views = [
                    np_out[r][a][:].rearrange("(g j p) d -> g p j d", j=2, p=P)
                    for a in range(4)
                ]
                npt = {}
                for a in range(4):
                    for g in range(8):
                        t = nppool.tile([P, 2, P], BF16, tag="np",
                                        name=f"r{r}_np{a}_{g}")
                        dma_engs[g % 2].dma_start(t[:], np_views[a][g])
                        npt[(a, g)] = t
                nk_order = [8 * g + 2 * a + j for a in range(4) for j in range(2)
                            for g in range(8)]
                for i, k in enumerate(nk_order):
                    a, j, g = (k % 8) // 2, (k % 8) % 2, k // 8
                    nc.tensor.matmul(p2c_ps[:], npt[(a, g)][:, j, :], rs[:, k, :],
                                     start=False, stop=(i == KN - 1))
                x_sb = wpool.tile([P, 512], BF16, tag="w512", name=f"r{r}_p2cs")
                nc.vector.tensor_copy(x_sb[:], p2c_ps[:])
                for g in range(4):
                    nc.tensor.matmul(gps[g][:], wvih[:, g * P : (g + 1) * P], x_sb[:],
                                     start=False, stop=True)
                lstm_half(gps, var_c, var_h, slice(0, VS), 8, f"r{r}_vl")

            # ===== vote MLP =====
            hv = var_h
            for li in range(2):
                ps = psA.tile([P, 512], F32, tag="mm", name=f"vo_l{li}")
                nc.tensor.matmul(ps[:], wvo[:, li, :], hv[:], start=True, stop=True)
                hn = wpool.tile([P, VS], BF16, tag="w512", name=f"vo_h{li}")
                nc.scalar.activation(hn[:], ps[:], AF.Relu, bias=bias[:, 12 + li : 13 + li])
                hv = hn
            ps = psA.tile([P, 512], F32, tag="mm", name="vo_l3")
            nc.tensor.matmul(ps[:1, :], wvo[:, 2, :1], hv[:], start=True, stop=True)
            yv = wpool.tile([P, 512], F32, tag="yv", name="vo_y")
            nc.scalar.activation(yv[:1, :], ps[:1, :], AF.Identity, bias=bias[:1, 14:15])
            nc.sync.dma_start(y_d[:], yv[:1, :])

    nc.compile()
    return nc


def _prep_inputs(unpack_rows, unpack_cols, params):
    """Host-side: densify unpack, build per-core shards + shared weights."""
    rows = np.asarray(unpack_rows).astype(np.int64)
    cols = np.asarray(unpack_cols).astype(np.int64)
    M = np.zeros((N_VAR, N_NODE), np.float32)
    np.add.at(M, (rows, cols), 1.0)
    deg_node = M.sum(axis=0)
    deg_var = M.sum(axis=1)

    def g(p, *ks):
        for k in ks:
            p = p[k]
        return np.asarray(p, np.float32)

    p = params
    w_vm = [g(p, "var_msg", l, "w") for l in ("l1", "l2", "l3")]
    b_vm = [g(p, "var_msg", l, "b") for l in ("l1", "l2", "l3")]
    w_nm = [g(p, "node_msg", l, "w") for l in ("l1", "l2", "l3")]
    b_nm = [g(p, "node_msg", l, "b") for l in ("l1", "l2", "l3")]
    # NOTE reference: node LSTM uses params['var_update'], var LSTM uses 'node_update'
    lu_n = {k: g(p, "var_update", k) for k in ("wih", "whh", "bih", "bhh")}
    lu_v = {k: g(p, "node_update", k) for k in ("wih", "whh", "bih", "bhh")}
    w_vo = [g(p, "node_vote", l, "w") for l in ("l1", "l2", "l3")]
    b_vo = [g(p, "node_vote", l, "b") for l in ("l1", "l2", "l3")]

    wmv = np.stack([w.T for w in w_vm], axis=1).astype(_nbf)  # [128,3,128]
    wmn = np.stack([w.T for w in w_nm], axis=1).astype(_nbf)
    wnih = lu_n["wih"].T.astype(_nbf)  # [128, 512]
    wnhh = lu_n["whh"].T.astype(_nbf)
    wvih = lu_v["wih"].T.astype(_nbf)
    wvhh = lu_v["whh"].T.astype(_nbf)
    w3v = np.zeros((DIM, DIM), np.float32)
    w3v[:, :1] = w_vo[2].T
    wvo = np.stack([w_vo[0].T, w_vo[1].T, w3v], axis=1).astype(_nbf)

    bias = np.zeros((P, 16), np.float32)
    bias[:, 0], bias[:, 1] = b_vm[0], b_vm[1]
    bias[:, 2], bias[:, 3] = b_nm[0], b_nm[1]
    bln = lu_n["bih"] + lu_n["bhh"]
    blv = lu_v["bih"] + lu_v["bhh"]
    for gi in range(4):
        bias[:, 4 + gi] = bln[gi * P : (gi + 1) * P]
        bias[:, 8 + gi] = blv[gi * P : (gi + 1) * P]
    bias[:, 12], bias[:, 13] = b_vo[0], b_vo[1]
    bias[0, 14] = b_vo[2][0]

    vh0 = g(p, "var_init", "w")[:, 0] + g(p, "var_init", "b")
    nh0 = g(p, "node_init", "w")[:, 0] + g(p, "node_init", "b")
    vh0_b = np.ascontiguousarray(np.broadcast_to(vh0[:, None], (P, VS))).astype(_nbf)
    nh0_b = np.ascontiguousarray(np.broadcast_to(nh0[:, None], (P, NS))).astype(_nbf)

    b3v_pad = np.zeros((P, P), np.float32)
    b3v_pad[0, :] = b_vm[2]
    b3n_pad = np.zeros((P, P), np.float32)
    b3n_pad[0, :] = b_nm[2]

    shared = {
        "wmv": wmv, "wmn": wmn,
        "wnih": wnih, "wnhh": wnhh, "wvih": wvih, "wvhh": wvhh,
        "wvo": wvo, "bias": bias,
        "vh0": vh0_b, "nh0": nh0_b,
        "b3v": b3v_pad.astype(_nbf), "b3n": b3n_pad.astype(_nbf),
    }
    in_maps = []
    for c in range(N_CORES):
        csd = np.zeros((KV + 1, P, NS), np.float32)
        csd[:KV] = M[:, c * NS : (c + 1) * NS].reshape(KV, P, NS)
        csd[KV, 0, :] = deg_node[c * NS : (c + 1) * NS]
        rsd = np.zeros((KN + 1, P, VS), np.float32)
        rsd[:KN] = np.ascontiguousarray(M[c * VS : (c + 1) * VS, :].T).reshape(KN, P, VS)
        rsd[KN, 0, :] = deg_var[c * VS : (c + 1) * VS]
        in_maps.append({"cs": csd.astype(_nbf), "rs": rsd.astype(_nbf), **shared})
    return in_maps


_CACHED = {}


def _get_nc():
    if "nc" not in _CACHED:
        _CACHED["nc"] = _build_nc()
    return _CACHED["nc"]


def kernel(unpack_rows, unpack_cols, params, _trace=False):
    in_maps = _prep_inputs(unpack_rows, unpack_cols, params)
    nc = _get_nc()
    res = run_bass_kernel_spmd(nc, in_maps, core_ids=list(range(N_CORES)), trace=_trace)
    out = np.concatenate(
        [np.asarray(res.results[c]["y"], np.float32).reshape(VS) for c in range(N_CORES)]
    )
    if _trace:
        _CACHED["last_results"] = res
    return out


# revision 24
# speedup vs baseline: 1.0797x; 1.0797x over previous
"""NeuroPredessor GNN message-passing kernel for 8 Trainium2 NeuronCores.

Sharding: core c owns vars [512c, 512c+512) and nodes [1024c, 1024c+1024).
The sparse unpack matrix is densified host-side to bf16 (integer counts are
exact in bf16) into two SBUF-resident shards per core:
  - cs: unpack[:, node_shard]            [4096, 1024]  (rhs of c2p SpMM)
  - rs: unpack[var_shard, :].T           [8192,  512]  (rhs of p2c SpMM)
Activations are feature-major [128(dim), tokens]; msg-MLP layer 3 emits
token-major tiles that are AllGathered (bf16) and feed the SpMM as lhsT.
The msg l3 bias is folded into the SpMM as one extra rank: b3 (x) deg.
"""

import sys

for _p in ("/opt/trn_rl_repo", "/root/.axon_site/_ro/trn_rl_repo"):
    if _p not in sys.path:
        sys.path.append(_p)

import numpy as np
import ml_dtypes

import concourse.bass as bass
import concourse.mybir as mybir
from concourse import tile, bacc
from concourse.bass_utils import run_bass_kernel_spmd

BF16 = mybir.dt.bfloat16
F32 = mybir.dt.float32
AF = mybir.ActivationFunctionType

N_VAR = 4096
N_NODE = 8192
DIM = 128
N_ROUNDS = 16
N_CORES = 8
VS = N_VAR // N_CORES  # 512 vars per core
NS = N_NODE // N_CORES  # 1024 nodes per core
KV = N_VAR // 128  # 32 var k-tiles
KN = N_NODE // 128  # 64 node k-tiles
P = 128

_nbf = ml_dtypes.bfloat16


def _build_nc(n_rounds=N_ROUNDS):
    nc = bacc.Bacc(None)

    # ---------------- external inputs ----------------
    cs_d = nc.dram_tensor("cs", [KV + 1, P, NS], BF16, kind="ExternalInput")
    rs_d = nc.dram_tensor("rs", [KN + 1, P, VS], BF16, kind="ExternalInput")
    wmv_d = nc.dram_tensor("wmv", [P, 3, P], BF16, kind="ExternalInput")
    wmn_d = nc.dram_tensor("wmn", [P, 3, P], BF16, kind="ExternalInput")
    wnih_d = nc.dram_tensor("wnih", [P, 4 * P], BF16, kind="ExternalInput")
    wnhh_d = nc.dram_tensor("wnhh", [P, 4 * P], BF16, kind="ExternalInput")
    wvih_d = nc.dram_tensor("wvih", [P, 4 * P], BF16, kind="ExternalInput")
    wvhh_d = nc.dram_tensor("wvhh", [P, 4 * P], BF16, kind="ExternalInput")
    wvo_d = nc.dram_tensor("wvo", [P, 3, P], BF16, kind="ExternalInput")
    b3v_d = nc.dram_tensor("b3v", [P, P], BF16, kind="ExternalInput")
    b3n_d = nc.dram_tensor("b3n", [P, P], BF16, kind="ExternalInput")
    bias_d = nc.dram_tensor("bias", [P, 16], F32, kind="ExternalInput")
    # bias cols: 0..1 var_msg b1,b2 | 2..3 node_msg b1,b2 | 4..7 node lstm i,f,g,o
    #            8..11 var lstm i,f,g,o | 12..13 vote b1,b2 | 14 vote b3 (row0)
    vh0_d = nc.dram_tensor("vh0", [P, VS], BF16, kind="ExternalInput")
    nh0_d = nc.dram_tensor("nh0", [P, NS], BF16, kind="ExternalInput")

    y_d = nc.dram_tensor("y", [1, VS], F32, kind="ExternalOutput")

    # ---------------- collective bounce buffers ----------------
    # var AG is split into two 512KB chunks (token halves) so nrt picks the
    # low-latency Mesh algorithm (<1MB) and chunk A's SpMM overlaps chunk B.
    vp_in = [
        [nc.dram_tensor(f"vp_in_{r}_{a}", [VS // 2, P], BF16) for a in range(2)]
        for r in range(n_rounds)
    ]
    vp_out = [
        [
            nc.dram_tensor(f"vp_out_{r}_{a}", [N_VAR // 2, P], BF16, addr_space="Shared")
            for a in range(2)
        ]
        for r in range(n_rounds)
    ]
    np_in = [nc.dram_tensor(f"np_in_{r}", [NS, P], BF16) for r in range(n_rounds)]
    np_out = [
        nc.dram_tensor(f"np_out_{r}", [N_NODE, P], BF16, addr_space="Shared")
        for r in range(n_rounds)
    ]

    RG = [list(range(N_CORES))]

    with tile.TileContext(nc) as tc:
        with (
            tc.tile_pool(name="const", bufs=1) as cpool,
            tc.tile_pool(name="state", bufs=1) as spool,
            tc.tile_pool(name="vp", bufs=16) as vppool,
            tc.tile_pool(name="npp", bufs=6) as nppool,
            tc.tile_pool(name="work", bufs=3) as wpool,
            tc.tile_pool(name="lstm", bufs=7) as lpool,
            tc.tile_pool(name="stage", bufs=6) as stpool,
            tc.tile_pool(name="psA", bufs=6, space="PSUM") as psA,
            tc.tile_pool(name="psB", bufs=2, space="PSUM") as psB,
        ):
            # ---------- resident loads ----------
            # Small weight/state loads go first (round 0 needs them); the big
            # unpack shards stream behind them, spread over two DMA queues.
            def cload(shape, dt, dram, nm):
                t = cpool.tile(shape, dt, name=nm)
                nc.sync.dma_start(t[:], dram[:])
                return t

            wmv = cload([P, 3, P], BF16, wmv_d, "wmv_sb")
            wmn = cload([P, 3, P], BF16, wmn_d, "wmn_sb")
            wnih = cload([P, 4 * P], BF16, wnih_d, "wnih_sb")
            wnhh = cload([P, 4 * P], BF16, wnhh_d, "wnhh_sb")
            wvih = cload([P, 4 * P], BF16, wvih_d, "wvih_sb")
            wvhh = cload([P, 4 * P], BF16, wvhh_d, "wvhh_sb")
            wvo = cload([P, 3, P], BF16, wvo_d, "wvo_sb")
            b3v = cload([P, P], BF16, b3v_d, "b3v_sb")
            b3n = cload([P, P], BF16, b3n_d, "b3n_sb")
            bias = cload([P, 16], F32, bias_d, "bias_sb")

            # persistent states (in-place updated)
            var_h = spool.tile([P, VS], BF16, name="var_h")
            nc.sync.dma_start(var_h[:], vh0_d[:])
            node_h = spool.tile([P, NS], BF16, name="node_h")
            nc.sync.dma_start(node_h[:], nh0_d[:])
            var_c = spool.tile([P, VS], F32, name="var_c")
            nc.vector.memset(var_c[:], 0.0)
            node_c = spool.tile([P, NS], F32, name="node_c")
            nc.vector.memset(node_c[:], 0.0)

            cs = cpool.tile([P, KV + 1, NS], BF16, name="cs_sb")
            for k in range(KV + 1):
                (nc.sync if k % 2 == 0 else nc.scalar).dma_start(cs[:, k, :], cs_d[k])
            rs = cpool.tile([P, KN + 1, VS], BF16, name="rs_sb")
            for k in range(KN + 1):
                (nc.sync if k % 2 == 0 else nc.scalar).dma_start(rs[:, k, :], rs_d[k])

            def mlp2(w, x, b0, ntok, nm):
                """Two relu layers, feature-major bf16 in/out."""
                h = x
                for li in range(2):
                    hn = wpool.tile([P, ntok], BF16, tag=f"w{ntok}", name=f"{nm}_h{li}")
                    for t0 in range(0, ntok, 512):
                        ps = psA.tile([P, 512], F32, tag="mm", name=f"{nm}_l{li}_{t0}")
                        nc.tensor.matmul(
                            ps[:], w[:, li, :], h[:, t0 : t0 + 512],
                            start=True, stop=True,
                        )
                        nc.scalar.activation(
                            hn[:, t0 : t0 + 512], ps[:], AF.Relu,
                            bias=bias[:, b0 + li : b0 + li + 1],
                        )
                    h = hn
                return h

            dma_engs = [nc.sync, nc.scalar]

            def msg_l3_to_bounce(w, h2, ntok, dram, nm):
                """l3 (no bias), token-major out, DMA into collective bounce."""
                for t in range(ntok // P):
                    ps = psB.tile([P, P], F32, tag="tok", name=f"{nm}_t{t}")
                    nc.tensor.matmul(
                        ps[:], h2[:, t * P : (t + 1) * P], w[:, 2, :],
                        start=True, stop=True,
                    )
                    st = stpool.tile([P, P], BF16, tag="st", name=f"{nm}_s{t}")
                    nc.scalar.activation(st[:], ps[:], AF.Copy)
                    dma_engs[t % 2].dma_start(dram[t * P : (t + 1) * P, :], st[:])

            def lstm_half(gps, c_st, h_st, hsl, b0, nm):
                """Gate psums [i,f,g,o] -> in-place update c_st/h_st slices."""
                i_s = lpool.tile([P, 512], F32, tag="ls", name=f"{nm}_i")
                f_s = lpool.tile([P, 512], F32, tag="ls", name=f"{nm}_f")
                g_t = lpool.tile([P, 512], F32, tag="ls", name=f"{nm}_g")
                o_s = lpool.tile([P, 512], F32, tag="ls", name=f"{nm}_o")
                nc.scalar.activation(i_s[:], gps[0][:], AF.Sigmoid, bias=bias[:, b0 : b0 + 1])
                nc.scalar.activation(f_s[:], gps[1][:], AF.Sigmoid, bias=bias[:, b0 + 1 : b0 + 2])
                nc.scalar.activation(g_t[:], gps[2][:], AF.Tanh, bias=bias[:, b0 + 2 : b0 + 3])
                nc.scalar.activation(o_s[:], gps[3][:], AF.Sigmoid, bias=bias[:, b0 + 3 : b0 + 4])
                t1 = lpool.tile([P, 512], F32, tag="ls", name=f"{nm}_t1")
                nc.vector.tensor_mul(t1[:], i_s[:], g_t[:])
                t2 = lpool.tile([P, 512], F32, tag="ls", name=f"{nm}_t2")
                nc.vector.tensor_mul(t2[:], f_s[:], c_st[:, hsl])
                nc.vector.tensor_add(c_st[:, hsl], t1[:], t2[:])
                tc2 = lpool.tile([P, 512], F32, tag="ls", name=f"{nm}_tc")
                nc.scalar.activation(tc2[:], c_st[:, hsl], AF.Tanh)
                nc.vector.tensor_mul(h_st[:, hsl], o_s[:], tc2[:])

            for r in range(n_rounds):
                # ===== var msg MLP + l3 -> bounce =====
                h2 = mlp2(wmv, var_h, 0, VS, f"r{r}_vm")
                for a in range(2):
                    for t in range(2):
                        ps = psB.tile([P, P], F32, tag="tok", name=f"r{r}_vm3_{a}{t}")
                        tt = 2 * a + t
                        nc.tensor.matmul(
                            ps[:], h2[:, tt * P : (tt + 1) * P], wmv[:, 2, :],
                            start=True, stop=True,
                        )
                        st = stpool.tile([P, P], BF16, tag="st", name=f"r{r}_vm3s_{a}{t}")
                        nc.scalar.activation(st[:], ps[:], AF.Copy)
                        dma_engs[t % 2].dma_start(vp_in[r][a][t * P : (t + 1) * P, :], st[:])
                    nc.gpsimd.collective_compute(
                        "AllGather", mybir.AluOpType.bypass, replica_groups=RG,
                        ins=[vp_in[r][a][:]], outs=[vp_out[r][a][:]],
                    )
                # ===== SpMM1 + node LSTM, sequential halves =====
                # Per half: bias-rank and Whh-part matmuls are emitted BEFORE
                # the k-loop so they sit ahead in the PE queue and execute
                # while the AllGather is still in flight. The k order consumes
                # AG chunk 0's tiles first so they overlap chunk 1's flight.
                vp_views = [
                    vp_out[r][a][:].rearrange("(g j p) d -> g p j d", j=2, p=P)
                    for a in range(2)
                ]
                k_order = [4 * g + 2 * a + j for a in range(2) for j in range(2)
                           for g in range(KV // 4)]
                vpt = None
                for h in range(2):
                    hsl = slice(h * 512, h * 512 + 512)
                    c2p = psA.tile([P, 512], F32, tag="mm", name=f"r{r}_c2p{h}")
                    nc.tensor.matmul(c2p[:], b3v[:], cs[:, KV, hsl],
                                     start=True, stop=False)
                    gps = []
                    for g in range(4):
                        ps = psA.tile([P, 512], F32, tag="mm", name=f"r{r}_ng{h}{g}")
                        nc.tensor.matmul(ps[:], wnhh[:, g * P : (g + 1) * P],
                                         node_h[:, hsl], start=True, stop=False)
                        gps.append(ps)
                    if vpt is None:
                        vpt = {}
                        for a in range(2):
                            for g in range(KV // 4):
                                t = vppool.tile([P, 2, P], BF16, tag="vp",
                                                name=f"r{r}_vp{a}_{g}")
                                dma_engs[g % 2].dma_start(t[:], vp_views[a][g])
                                vpt[(a, g)] = t
                    for i, k in enumerate(k_order):
                        a, j, g = (k % 4) // 2, (k % 4) % 2, k // 4
                        nc.tensor.matmul(c2p[:], vpt[(a, g)][:, j, :],
                                         cs[:, k, hsl],
                                         start=False, stop=(i == KV - 1))
                    x_sb = wpool.tile([P, 512], BF16, tag="w512", name=f"r{r}_c2ps{h}")
                    nc.vector.tensor_copy(x_sb[:], c2p[:])
                    for g in range(4):
                        nc.tensor.matmul(gps[g][:], wnih[:, g * P : (g + 1) * P],
                                         x_sb[:], start=False, stop=True)
                    lstm_half(gps, node_c, node_h, hsl, 4, f"r{r}_nl{h}")

                # ===== node msg MLP + l3 -> bounce =====
                h2n = mlp2(wmn, node_h, 2, NS, f"r{r}_nm")
                msg_l3_to_bounce(wmn, h2n, NS, np_in[r], f"r{r}_nm3")

                # ===== AllGather node_pre =====
                nc.gpsimd.collective_compute(
                    "AllGather", mybir.AluOpType.bypass, replica_groups=RG,
                    ins=[np_in[r][:]], outs=[np_out[r][:]],
                )
                # ===== SpMM2 + var LSTM (bias/Whh emitted first for AG overlap) =====
                p2c_ps = psA.tile([P, 512], F32, tag="mm", name=f"r{r}_p2c")
                nc.tensor.matmul(p2c_ps[:], b3n[:], rs[:, KN, :],
                                 start=True, stop=False)
                gps = []
                for g in range(4):
                    ps = psA.tile([P, 512], F32, tag="mm", name=f"r{r}_vg{g}")
                    nc.tensor.matmul(ps[:], wvhh[:, g * P : (g + 1) * P], var_h[:],
                                     start=True, stop=False)
                    gps.append(ps)
                np_view = np_out[r][:].rearrange("(g j p) d -> g p j d", j=4, p=P)
                npt = []
                for g in range(KN // 4):
                    t = nppool.tile([P, 4, P], BF16, tag="np", name=f"r{r}_np{g}")
                    dma_engs[g % 2].dma_start(t[:], np_view[g])
                    npt.append(t)
                for k in range(KN):
                    nc.tensor.matmul(p2c_ps[:], npt[k // 4][:, k % 4, :], rs[:, k, :],
                                     start=False, stop=(k == KN - 1))
                x_sb = wpool.tile([P, 512], BF16, tag="w512", name=f"r{r}_p2cs")
                nc.vector.tensor_copy(x_sb[:], p2c_ps[:])
                for g in range(4):
                    nc.tensor.matmul(gps[g][:], wvih[:, g * P : (g + 1) * P], x_sb[:],
                                     start=False, stop=True)
                lstm_half(gps, var_c, var_h, slice(0, VS), 8, f"r{r}_vl")

            # ===== vote MLP =====
            hv = var_h
            for li in range(2):
                ps = psA.tile([P, 512], F32, tag="mm", name=f"vo_l{li}")
                nc.tensor.matmul(ps[:], wvo[:, li, :], hv[:], start=True, stop=True)
                hn = wpool.tile([P, VS], BF16, tag="w512", name=f"vo_h{li}")
                nc.scalar.activation(hn[:], ps[:], AF.Relu, bias=bias[:, 12 + li : 13 + li])
                hv = hn
            ps = psA.tile([P, 512], F32, tag="mm", name="vo_l3")
            nc.tensor.matmul(ps[:1, :], wvo[:, 2, :1], hv[:], start=True, stop=True)
            yv = wpool.tile([P, 512], F32, tag="yv", name="vo_y")
            nc.scalar.activation(yv[:1, :], ps[:1, :], AF.Identity, bias=bias[:1, 14:15])
            nc.sync.dma_start(y_d[:], yv[:1, :])

    nc.compile()
    return nc


def _prep_inputs(unpack_rows, unpack_cols, params):
    """Host-side: densify unpack, build per-core shards + shared weights."""
    rows = np.asarray(unpack_rows).astype(np.int64)
    cols = np.asarray(unpack_cols).astype(np.int64)
    M = np.zeros((N_VAR, N_NODE), np.float32)
    np.add.at(M, (rows, cols), 1.0)
    deg_node = M.sum(axis=0)
    deg_var = M.sum(axis=1)

    def g(p, *ks):
        for k in ks:
            p = p[k]
        return np.asarray(p, np.float32)

    p = params
    w_vm = [g(p, "var_msg", l, "w") for l in ("l1", "l2", "l3")]
    b_vm = [g(p, "var_msg", l, "b") for l in ("l1", "l2", "l3")]
    w_nm = [g(p, "node_msg", l, "w") for l in ("l1", "l2", "l3")]
    b_nm = [g(p, "node_msg", l, "b") for l in ("l1", "l2", "l3")]
    # NOTE reference: node LSTM uses params['var_update'], var LSTM uses 'node_update'
    lu_n = {k: g(p, "var_update", k) for k in ("wih", "whh", "bih", "bhh")}
    lu_v = {k: g(p, "node_update", k) for k in ("wih", "whh", "bih", "bhh")}
    w_vo = [g(p, "node_vote", l, "w") for l in ("l1", "l2", "l3")]
    b_vo = [g(p, "node_vote", l, "b") for l in ("l1", "l2", "l3")]

    wmv = np.stack([w.T for w in w_vm], axis=1).astype(_nbf)  # [128,3,128]
    wmn = np.stack([w.T for w in w_nm], axis=1).astype(_nbf)
    wnih = lu_n["wih"].T.astype(_nbf)  # [128, 512]
    wnhh = lu_n["whh"].T.astype(_nbf)
    wvih = lu_v["wih"].T.astype(_nbf)
    wvhh = lu_v["whh"].T.astype(_nbf)
    w3v = np.zeros((DIM, DIM), np.float32)
    w3v[:, :1] = w_vo[2].T
    wvo = np.stack([w_vo[0].T, w_vo[1].T, w3v], axis=1).astype(_nbf)

    bias = np.zeros((P, 16), np.float32)
    bias[:, 0], bias[:, 1] = b_vm[0], b_vm[1]
    bias[:, 2], bias[:, 3] = b_nm[0], b_nm[1]
    bln = lu_n["bih"] + lu_n["bhh"]
    blv = lu_v["bih"] + lu_v["bhh"]
    for gi in range(4):
        bias[:, 4 + gi] = bln[gi * P : (gi + 1) * P]
        bias[:, 8 + gi] = blv[gi * P : (gi + 1) * P]
    bias[:, 12], bias[:, 13] = b_vo[0], b_vo[1]
    bias[0, 14] = b_vo[2][0]

    vh0 = g(p, "var_init", "w")[:, 0] + g(p, "var_init", "b")
    nh0 = g(p, "node_init", "w")[:, 0] + g(p, "node_init", "b")
    vh0_b = np.ascontiguousarray(np.broadcast_to(vh0[:, None], (P, VS))).astype(_nbf)
    nh0_b = np.ascontiguousarray(np.broadcast_to(nh0[:, None], (P, NS))).astype(_nbf)

    b3v_pad = np.zeros((P, P), np.float32)
    b3v_pad[0, :] = b_vm[2]
    b3n_pad = np.zeros((P, P), np.float32)
    b3n_pad[0, :] = b_nm[2]

    shared = {
        "wmv": wmv, "wmn": wmn,
        "wnih": wnih, "wnhh": wnhh, "wvih": wvih, "wvhh": wvhh,
        "wvo": wvo, "bias": bias,
        "vh0": vh0_b, "nh0": nh0_b,
        "b3v": b3v_pad.astype(_nbf), "b3n": b3n_pad.astype(_nbf),
    }
    in_maps = []
    for c in range(N_CORES):
        csd = np.zeros((KV + 1, P, NS), np.float32)
        csd[:KV] = M[:, c * NS : (c + 1) * NS].reshape(KV, P, NS)
        csd[KV, 0, :] = deg_node[c * NS : (c + 1) * NS]
        rsd = np.zeros((KN + 1, P, VS), np.float32)
        rsd[:KN] = np.ascontiguousarray(M[c * VS : (c + 1) * VS, :].T).reshape(KN, P, VS)
        rsd[KN, 0, :] = deg_var[c * VS : (c + 1) * VS]
        in_maps.append({"cs": csd.astype(_nbf), "rs": rsd.astype(_nbf), **shared})
    return in_maps


_CACHED = {}


def _get_nc():
    if "nc" not in _CACHED:
        _CACHED["nc"] = _build_nc()
    return _CACHED["nc"]


def kernel(unpack_rows, unpack_cols, params, _trace=False):
    in_maps = _prep_inputs(unpack_rows, unpack_cols, params)
    nc = _get_nc()
    res = run_bass_kernel_spmd(nc, in_maps, core_ids=list(range(N_CORES)), trace=_trace)
    out = np.concatenate(
        [np.asarray(res.results[c]["y"], np.float32).reshape(VS) for c in range(N_CORES)]
    )
    if _trace:
        _CACHED["last_results"] = res
    return out


# revision 28
# speedup vs baseline: 1.0994x; 1.0183x over previous
"""NeuroPredessor GNN message-passing kernel for 8 Trainium2 NeuronCores.

Sharding: core c owns vars [512c, 512c+512) and nodes [1024c, 1024c+1024).
The sparse unpack matrix is densified host-side to bf16 (integer counts are
exact in bf16) into two SBUF-resident shards per core:
  - cs: unpack[:, node_shard]            [4096, 1024]  (rhs of c2p SpMM)
  - rs: unpack[var_shard, :].T           [8192,  512]  (rhs of p2c SpMM)
Activations are feature-major [128(dim), tokens]; msg-MLP layer 3 emits
token-major tiles that are AllGathered (bf16) and feed the SpMM as lhsT.
The msg l3 bias is folded into the SpMM as one extra rank: b3 (x) deg.
"""

import sys

for _p in ("/opt/trn_rl_repo", "/root/.axon_site/_ro/trn_rl_repo"):
    if _p not in sys.path:
        sys.path.append(_p)

import numpy as np
import ml_dtypes

import concourse.bass as bass
import concourse.mybir as mybir
from concourse import tile, bacc
from concourse.bass_utils import run_bass_kernel_spmd

BF16 = mybir.dt.bfloat16
F32 = mybir.dt.float32
AF = mybir.ActivationFunctionType

N_VAR = 4096
N_NODE = 8192
DIM = 128
N_ROUNDS = 16
N_CORES = 8
VS = N_VAR // N_CORES  # 512 vars per core
NS = N_NODE // N_CORES  # 1024 nodes per core
KV = N_VAR // 128  # 32 var k-tiles
KN = N_NODE // 128  # 64 node k-tiles
P = 128

_nbf = ml_dtypes.bfloat16


def _build_nc(n_rounds=N_ROUNDS):
    nc = bacc.Bacc(None)

    # ---------------- external inputs ----------------
    cs_d = nc.dram_tensor("cs", [KV + 1, P, NS], BF16, kind="ExternalInput")
    rs_d = nc.dram_tensor("rs", [KN + 1, P, VS], BF16, kind="ExternalInput")
    wmv_d = nc.dram_tensor("wmv", [P, 3, P], BF16, kind="ExternalInput")
    wmn_d = nc.dram_tensor("wmn", [P, 3, P], BF16, kind="ExternalInput")
    wnih_d = nc.dram_tensor("wnih", [P, 4 * P], BF16, kind="ExternalInput")
    wnhh_d = nc.dram_tensor("wnhh", [P, 4 * P], BF16, kind="ExternalInput")
    wvih_d = nc.dram_tensor("wvih", [P, 4 * P], BF16, kind="ExternalInput")
    wvhh_d = nc.dram_tensor("wvhh", [P, 4 * P], BF16, kind="ExternalInput")
    wvo_d = nc.dram_tensor("wvo", [P, 3, P], BF16, kind="ExternalInput")
    b3v_d = nc.dram_tensor("b3v", [P, P], BF16, kind="ExternalInput")
    b3n_d = nc.dram_tensor("b3n", [P, P], BF16, kind="ExternalInput")
    bias_d = nc.dram_tensor("bias", [P, 16], F32, kind="ExternalInput")
    # bias cols: 0..1 var_msg b1,b2 | 2..3 node_msg b1,b2 | 4..7 node lstm i,f,g,o
    #            8..11 var lstm i,f,g,o | 12..13 vote b1,b2 | 14 vote b3 (row0)
    vh0_d = nc.dram_tensor("vh0", [P, VS], BF16, kind="ExternalInput")
    nh0_d = nc.dram_tensor("nh0", [P, NS], BF16, kind="ExternalInput")

    y_d = nc.dram_tensor("y", [1, VS], F32, kind="ExternalOutput")

    # ---------------- collective bounce buffers ----------------
    # var AG is split into two 512KB chunks (token halves) so nrt picks the
    # low-latency Mesh algorithm (<1MB) and chunk A's SpMM overlaps chunk B.
    vp_in = [
        [nc.dram_tensor(f"vp_in_{r}_{a}", [VS // 2, P], BF16) for a in range(2)]
        for r in range(n_rounds)
    ]
    vp_out = [
        [
            nc.dram_tensor(f"vp_out_{r}_{a}", [N_VAR // 2, P], BF16, addr_space="Shared")
            for a in range(2)
        ]
        for r in range(n_rounds)
    ]
    np_in = [nc.dram_tensor(f"np_in_{r}", [NS, P], BF16) for r in range(n_rounds)]
    np_out = [
        nc.dram_tensor(f"np_out_{r}", [N_NODE, P], BF16, addr_space="Shared")
        for r in range(n_rounds)
    ]

    RG = [list(range(N_CORES))]

    with tile.TileContext(nc) as tc:
        with (
            tc.tile_pool(name="const", bufs=1) as cpool,
            tc.tile_pool(name="state", bufs=1) as spool,
            tc.tile_pool(name="vp", bufs=16) as vppool,
            tc.tile_pool(name="npp", bufs=6) as nppool,
            tc.tile_pool(name="work", bufs=3) as wpool,
            tc.tile_pool(name="lstm", bufs=7) as lpool,
            tc.tile_pool(name="stage", bufs=6) as stpool,
            tc.tile_pool(name="psA", bufs=6, space="PSUM") as psA,
            tc.tile_pool(name="psB", bufs=2, space="PSUM") as psB,
        ):
            # ---------- resident loads ----------
            # Small weight/state loads go first (round 0 needs them); the big
            # unpack shards stream behind them, spread over two DMA queues.
            def cload(shape, dt, dram, nm):
                t = cpool.tile(shape, dt, name=nm)
                nc.sync.dma_start(t[:], dram[:])
                return t

            wmv = cload([P, 3, P], BF16, wmv_d, "wmv_sb")
            wmn = cload([P, 3, P], BF16, wmn_d, "wmn_sb")
            wnih = cload([P, 4 * P], BF16, wnih_d, "wnih_sb")
            wnhh = cload([P, 4 * P], BF16, wnhh_d, "wnhh_sb")
            wvih = cload([P, 4 * P], BF16, wvih_d, "wvih_sb")
            wvhh = cload([P, 4 * P], BF16, wvhh_d, "wvhh_sb")
            wvo = cload([P, 3, P], BF16, wvo_d, "wvo_sb")
            b3v = cload([P, P], BF16, b3v_d, "b3v_sb")
            b3n = cload([P, P], BF16, b3n_d, "b3n_sb")
            bias = cload([P, 16], F32, bias_d, "bias_sb")

            # persistent states (in-place updated)
            var_h = spool.tile([P, VS], BF16, name="var_h")
            nc.sync.dma_start(var_h[:], vh0_d[:])
            node_h = spool.tile([P, NS], BF16, name="node_h")
            nc.sync.dma_start(node_h[:], nh0_d[:])
            var_c = spool.tile([P, VS], F32, name="var_c")
            nc.vector.memset(var_c[:], 0.0)
            node_c = spool.tile([P, NS], F32, name="node_c")
            nc.vector.memset(node_c[:], 0.0)

            # Bulk shard loads are de-prioritized so round-0's stage DMAs and
            # ACT work are not queued behind 98 DMA issues.
            cs = cpool.tile([P, KV + 1, NS], BF16, name="cs_sb")
            rs = cpool.tile([P, KN + 1, VS], BF16, name="rs_sb")
            with tc.high_priority(offset=-10_000_000):
                for k in range(KV + 1):
                    (nc.sync if k % 2 == 0 else nc.scalar).dma_start(cs[:, k, :], cs_d[k])
                for k in range(KN + 1):
                    (nc.sync if k % 2 == 0 else nc.scalar).dma_start(rs[:, k, :], rs_d[k])

            def mlp2(w, x, b0, ntok, nm):
                """Two relu layers, feature-major bf16 in/out."""
                h = x
                for li in range(2):
                    hn = wpool.tile([P, ntok], BF16, tag=f"w{ntok}", name=f"{nm}_h{li}")
                    for t0 in range(0, ntok, 512):
                        ps = psA.tile([P, 512], F32, tag="mm", name=f"{nm}_l{li}_{t0}")
                        nc.tensor.matmul(
                            ps[:], w[:, li, :], h[:, t0 : t0 + 512],
                            start=True, stop=True,
                        )
                        # relu(x+b) on DVE keeps the ACT engine free for the
                        # LSTM sigmoid/tanh chains and stage casts.
                        nc.vector.tensor_scalar(
                            hn[:, t0 : t0 + 512], ps[:],
                            bias[:, b0 + li : b0 + li + 1], 0.0,
                            mybir.AluOpType.add, mybir.AluOpType.max,
                        )
                    h = hn
                return h

            dma_engs = [nc.sync, nc.scalar]

            def msg_l3_to_bounce(w, h2, ntok, dram, nm):
                """l3 (no bias), token-major out, DMA into collective bounce."""
                for t in range(ntok // P):
                    ps = psB.tile([P, P], F32, tag="tok", name=f"{nm}_t{t}")
                    nc.tensor.matmul(
                        ps[:], h2[:, t * P : (t + 1) * P], w[:, 2, :],
                        start=True, stop=True,
                    )
                    st = stpool.tile([P, P], BF16, tag="st", name=f"{nm}_s{t}")
                    nc.vector.tensor_copy(st[:], ps[:])
                    dma_engs[t % 2].dma_start(dram[t * P : (t + 1) * P, :], st[:])

            def lstm_half(gps, c_st, h_st, hsl, b0, nm):
                """Gate psums [i,f,g,o] -> in-place update c_st/h_st slices."""
                i_s = lpool.tile([P, 512], F32, tag="ls", name=f"{nm}_i")
                f_s = lpool.tile([P, 512], F32, tag="ls", name=f"{nm}_f")
                g_t = lpool.tile([P, 512], F32, tag="ls", name=f"{nm}_g")
                o_s = lpool.tile([P, 512], F32, tag="ls", name=f"{nm}_o")
                nc.scalar.activation(i_s[:], gps[0][:], AF.Sigmoid, bias=bias[:, b0 : b0 + 1])
                nc.scalar.activation(f_s[:], gps[1][:], AF.Sigmoid, bias=bias[:, b0 + 1 : b0 + 2])
                nc.scalar.activation(g_t[:], gps[2][:], AF.Tanh, bias=bias[:, b0 + 2 : b0 + 3])
                nc.scalar.activation(o_s[:], gps[3][:], AF.Sigmoid, bias=bias[:, b0 + 3 : b0 + 4])
                t1 = lpool.tile([P, 512], F32, tag="ls", name=f"{nm}_t1")
                nc.vector.tensor_mul(t1[:], i_s[:], g_t[:])
                t2 = lpool.tile([P, 512], F32, tag="ls", name=f"{nm}_t2")
                nc.vector.tensor_mul(t2[:], f_s[:], c_st[:, hsl])
                nc.vector.tensor_add(c_st[:, hsl], t1[:], t2[:])
                tc2 = lpool.tile([P, 512], F32, tag="ls", name=f"{nm}_tc")
                nc.scalar.activation(tc2[:], c_st[:, hsl], AF.Tanh)
                nc.vector.tensor_mul(h_st[:, hsl], o_s[:], tc2[:])

            for r in range(n_rounds):
                # ===== var msg MLP + l3 -> bounce =====
                h2 = mlp2(wmv, var_h, 0, VS, f"r{r}_vm")
                for a in range(2):
                    for t in range(2):
                        ps = psB.tile([P, P], F32, tag="tok", name=f"r{r}_vm3_{a}{t}")
                        tt = 2 * a + t
                        nc.tensor.matmul(
                            ps[:], h2[:, tt * P : (tt + 1) * P], wmv[:, 2, :],
                            start=True, stop=True,
                        )
                        st = stpool.tile([P, P], BF16, tag="st", name=f"r{r}_vm3s_{a}{t}")
                        nc.vector.tensor_copy(st[:], ps[:])
                        dma_engs[t % 2].dma_start(vp_in[r][a][t * P : (t + 1) * P, :], st[:])
                    nc.gpsimd.collective_compute(
                        "AllGather", mybir.AluOpType.bypass, replica_groups=RG,
                        ins=[vp_in[r][a][:]], outs=[vp_out[r][a][:]],
                    )
                # ===== SpMM1 + node LSTM, sequential halves =====
                # Per half: bias-rank and Whh-part matmuls are emitted BEFORE
                # the k-loop so they sit ahead in the PE queue and execute
                # while the AllGather is still in flight. The k order consumes
                # AG chunk 0's tiles first so they overlap chunk 1's flight.
                vp_views = [
                    vp_out[r][a][:].rearrange("(g j p) d -> g p j d", j=2, p=P)
                    for a in range(2)
                ]
                k_order = [4 * g + 2 * a + j for a in range(2) for j in range(2)
                           for g in range(KV // 4)]
                vpt = None
                for h in range(2):
                    hsl = slice(h * 512, h * 512 + 512)
                    c2p = psA.tile([P, 512], F32, tag="mm", name=f"r{r}_c2p{h}")
                    nc.tensor.matmul(c2p[:], b3v[:], cs[:, KV, hsl],
                                     start=True, stop=False)
                    gps = []
                    for g in range(4):
                        ps = psA.tile([P, 512], F32, tag="mm", name=f"r{r}_ng{h}{g}")
                        nc.tensor.matmul(ps[:], wnhh[:, g * P : (g + 1) * P],
                                         node_h[:, hsl], start=True, stop=False)
                        gps.append(ps)
                    if vpt is None:
                        vpt = {}
                        for a in range(2):
                            for g in range(KV // 4):
                                t = vppool.tile([P, 2, P], BF16, tag="vp",
                                                name=f"r{r}_vp{a}_{g}")
                                dma_engs[g % 2].dma_start(t[:], vp_views[a][g])
                                vpt[(a, g)] = t
                    for i, k in enumerate(k_order):
                        a, j, g = (k % 4) // 2, (k % 4) % 2, k // 4
                        nc.tensor.matmul(c2p[:], vpt[(a, g)][:, j, :],
                                         cs[:, k, hsl],
                                         start=False, stop=(i == KV - 1))
                    x_sb = wpool.tile([P, 512], BF16, tag="w512", name=f"r{r}_c2ps{h}")
                    nc.vector.tensor_copy(x_sb[:], c2p[:])
                    for g in range(4):
                        nc.tensor.matmul(gps[g][:], wnih[:, g * P : (g + 1) * P],
                                         x_sb[:], start=False, stop=True)
                    lstm_half(gps, node_c, node_h, hsl, 4, f"r{r}_nl{h}")

                # ===== node msg MLP + l3 -> bounce =====
                h2n = mlp2(wmn, node_h, 2, NS, f"r{r}_nm")
                msg_l3_to_bounce(wmn, h2n, NS, np_in[r], f"r{r}_nm3")

                # ===== AllGather node_pre =====
                nc.gpsimd.collective_compute(
                    "AllGather", mybir.AluOpType.bypass, replica_groups=RG,
                    ins=[np_in[r][:]], outs=[np_out[r][:]],
                )
                # ===== SpMM2 + var LSTM (bias/Whh emitted first for AG overlap) =====
                p2c_ps = psA.tile([P, 512], F32, tag="mm", name=f"r{r}_p2c")
                nc.tensor.matmul(p2c_ps[:], b3n[:], rs[:, KN, :],
                                 start=True, stop=False)
                gps = []
                for g in range(4):
                    ps = psA.tile([P, 512], F32, tag="mm", name=f"r{r}_vg{g}")
                    nc.tensor.matmul(ps[:], wvhh[:, g * P : (g + 1) * P], var_h[:],
                                     start=True, stop=False)
                    gps.append(ps)
                np_view = np_out[r][:].rearrange("(g j p) d -> g p j d", j=4, p=P)
                npt = []
                for g in range(KN // 4):
                    t = nppool.tile([P, 4, P], BF16, tag="np", name=f"r{r}_np{g}")
                    dma_engs[g % 2].dma_start(t[:], np_view[g])
                    npt.append(t)
                for k in range(KN):
                    nc.tensor.matmul(p2c_ps[:], npt[k // 4][:, k % 4, :], rs[:, k, :],
                                     start=False, stop=(k == KN - 1))
                x_sb = wpool.tile([P, 512], BF16, tag="w512", name=f"r{r}_p2cs")
                nc.vector.tensor_copy(x_sb[:], p2c_ps[:])
                for g in range(4):
                    nc.tensor.matmul(gps[g][:], wvih[:, g * P : (g + 1) * P], x_sb[:],
                                     start=False, stop=True)
                lstm_half(gps, var_c, var_h, slice(0, VS), 8, f"r{r}_vl")

            # ===== vote MLP =====
            hv = var_h
            for li in range(2):
                ps = psA.tile([P, 512], F32, tag="mm", name=f"vo_l{li}")
                nc.tensor.matmul(ps[:], wvo[:, li, :], hv[:], start=True, stop=True)
                hn = wpool.tile([P, VS], BF16, tag="w512", name=f"vo_h{li}")
                nc.scalar.activation(hn[:], ps[:], AF.Relu, bias=bias[:, 12 + li : 13 + li])
                hv = hn
            ps = psA.tile([P, 512], F32, tag="mm", name="vo_l3")
            nc.tensor.matmul(ps[:1, :], wvo[:, 2, :1], hv[:], start=True, stop=True)
            yv = wpool.tile([P, 512], F32, tag="yv", name="vo_y")
            nc.scalar.activation(yv[:1, :], ps[:1, :], AF.Identity, bias=bias[:1, 14:15])
            nc.sync.dma_start(y_d[:], yv[:1, :])

    nc.compile()
    return nc


def _prep_inputs(unpack_rows, unpack_cols, params):
    """Host-side: densify unpack, build per-core shards + shared weights."""
    rows = np.asarray(unpack_rows).astype(np.int64)
    cols = np.asarray(unpack_cols).astype(np.int64)
    M = np.zeros((N_VAR, N_NODE), np.float32)
    np.add.at(M, (rows, cols), 1.0)
    deg_node = M.sum(axis=0)
    deg_var = M.sum(axis=1)

    def g(p, *ks):
        for k in ks:
            p = p[k]
        return np.asarray(p, np.float32)

    p = params
    w_vm = [g(p, "var_msg", l, "w") for l in ("l1", "l2", "l3")]
    b_vm = [g(p, "var_msg", l, "b") for l in ("l1", "l2", "l3")]
    w_nm = [g(p, "node_msg", l, "w") for l in ("l1", "l2", "l3")]
    b_nm = [g(p, "node_msg", l, "b") for l in ("l1", "l2", "l3")]
    # NOTE reference: node LSTM uses params['var_update'], var LSTM uses 'node_update'
    lu_n = {k: g(p, "var_update", k) for k in ("wih", "whh", "bih", "bhh")}
    lu_v = {k: g(p, "node_update", k) for k in ("wih", "whh", "bih", "bhh")}
    w_vo = [g(p, "node_vote", l, "w") for l in ("l1", "l2", "l3")]
    b_vo = [g(p, "node_vote", l, "b") for l in ("l1", "l2", "l3")]

    wmv = np.stack([w.T for w in w_vm], axis=1).astype(_nbf)  # [128,3,128]
    wmn = np.stack([w.T for w in w_nm], axis=1).astype(_nbf)
    wnih = lu_n["wih"].T.astype(_nbf)  # [128, 512]
    wnhh = lu_n["whh"].T.astype(_nbf)
    wvih = lu_v["wih"].T.astype(_nbf)
    wvhh = lu_v["whh"].T.astype(_nbf)
    w3v = np.zeros((DIM, DIM), np.float32)
    w3v[:, :1] = w_vo[2].T
    wvo = np.stack([w_vo[0].T, w_vo[1].T, w3v], axis=1).astype(_nbf)

    bias = np.zeros((P, 16), np.float32)
    bias[:, 0], bias[:, 1] = b_vm[0], b_vm[1]
    bias[:, 2], bias[:, 3] = b_nm[0], b_nm[1]
    bln = lu_n["bih"] + lu_n["bhh"]
    blv = lu_v["bih"] + lu_v["bhh"]
    for gi in range(4):
        bias[:, 4 + gi] = bln[gi * P : (gi + 1) * P]
        bias[:, 8 + gi] = blv[gi * P : (gi + 1) * P]
    bias[:, 12], bias[:, 13] = b_vo[0], b_vo[1]
    bias[0, 14] = b_vo[2][0]

    vh0 = g(p, "var_init", "w")[:, 0] + g(p, "var_init", "b")
    nh0 = g(p, "node_init", "w")[:, 0] + g(p, "node_init", "b")
    vh0_b = np.ascontiguousarray(np.broadcast_to(vh0[:, None], (P, VS))).astype(_nbf)
    nh0_b = np.ascontiguousarray(np.broadcast_to(nh0[:, None], (P, NS))).astype(_nbf)

    b3v_pad = np.zeros((P, P), np.float32)
    b3v_pad[0, :] = b_vm[2]
    b3n_pad = np.zeros((P, P), np.float32)
    b3n_pad[0, :] = b_nm[2]

    shared = {
        "wmv": wmv, "wmn": wmn,
        "wnih": wnih, "wnhh": wnhh, "wvih": wvih, "wvhh": wvhh,
        "wvo": wvo, "bias": bias,
        "vh0": vh0_b, "nh0": nh0_b,
        "b3v": b3v_pad.astype(_nbf), "b3n": b3n_pad.astype(_nbf),
    }
    in_maps = []
    for c in range(N_CORES):
        csd = np.zeros((KV + 1, P, NS), np.float32)
        csd[:KV] = M[:, c * NS : (c + 1) * NS].reshape(KV, P, NS)
        csd[KV, 0, :] = deg_node[c * NS : (c + 1) * NS]
        rsd = np.zeros((KN + 1, P, VS), np.float32)
        rsd[:KN] = np.ascontiguousarray(M[c * VS : (c + 1) * VS, :].T).reshape(KN, P, VS)
        rsd[KN, 0, :] = deg_var[c * VS : (c + 1) * VS]
        in_maps.append({"cs": csd.astype(_nbf), "rs": rsd.astype(_nbf), **shared})
    return in_maps


_CACHED = {}


def _get_nc():
    if "nc" not in _CACHED:
        _CACHED["nc"] = _build_nc()
    return _CACHED["nc"]


def kernel(unpack_rows, unpack_cols, params, _trace=False):
    in_maps = _prep_inputs(unpack_rows, unpack_cols, params)
    nc = _get_nc()
    res = run_bass_kernel_spmd(nc, in_maps, core_ids=list(range(N_CORES)), trace=_trace)
    out = np.concatenate(
        [np.asarray(res.results[c]["y"], np.float32).reshape(VS) for c in range(N_CORES)]
    )
    if _trace:
        _CACHED["last_results"] = res
    return out
